# revision 13
# baseline (speedup 1.0000x reference)
"""Trainium2 Bass kernel for chunked gated linear RNN (nn_ChunkRnn).

Model (reference semantics):
    proj = x @ W_in; q, f_raw, v = split(proj)
    q = silu(q); f = sigmoid(f_raw); log_f = log_sigmoid(f); k = 1 - f
    per head (d=64): S_t = diag(exp(log_f_t)) S_{t-1} + k_t v_t^T; o_t = q_t^T S_t
    out = layernorm(o) @ W_out

Sharding: 8 cores = 2 (batch) x 4 (head groups of 4 heads).
Each core: in_proj for its 4 heads (q,f transposed layout; v natural),
chunk-parallel RNN (CHUNK=128), 8-core AllToAll to exchange head outputs,
then layernorm+out_proj for its 1024-token slice (layernorm folded into
host-premultiplied W' = norm_w*W_out plus a colsum correction term).
"""
import sys

sys.path.insert(0, "/opt/trn_rl_repo")

import numpy as np

import concourse.bass as bass
import concourse.mybir as mybir
import concourse.tile as tile
from concourse import bacc
from concourse.bass import ts, ds
from concourse.bass_utils import run_bass_kernel_spmd
from concourse.masks import make_upper_triangular, make_identity

F32 = mybir.dt.float32
MM_DT = mybir.dt.bfloat16  # matmul operand dtype (bfloat16 | float32r | float32)

N_CORES = 8
GROUPS8 = [[0, 1, 2, 3, 4, 5, 6, 7]]
P = 128
D = 1024          # embed dim
T = 4096          # tokens per core (one batch element)
C = 128           # chunk length (exact reformulation; reference uses 64)
NCH = T // C      # 32 chunks
NH = 4            # heads per core
HD = 64           # head dim
KT = D // P       # 8 k-tiles
SBT = 512         # superblock tokens for in_proj
NSB = T // SBT    # 8 superblocks
HCOL = NH * HD    # 256 local proj cols per q/f/v
EPS = 1e-5

AFT = mybir.ActivationFunctionType
ALU = mybir.AluOpType


def build_program():
    nc = bacc.Bacc("TRN2", target_bir_lowering=False, debug=False,
                   num_devices=N_CORES)

    xl = nc.dram_tensor("xl", [T, D], F32, kind="ExternalInput")
    wq = nc.dram_tensor("wq", [D, HCOL], F32, kind="ExternalInput")
    wf = nc.dram_tensor("wf", [D, HCOL], F32, kind="ExternalInput")
    wv = nc.dram_tensor("wv", [D, HCOL], F32, kind="ExternalInput")
    wpo = nc.dram_tensor("wpo", [D, D], F32, kind="ExternalInput")  # norm_w[:,None]*W_out
    csum = nc.dram_tensor("csum", [1, D], F32, kind="ExternalInput")  # wpo.sum(0)
    sel = nc.dram_tensor("sel", [1, 2], F32, kind="ExternalInput")  # [lo, hi] group select
    out = nc.dram_tensor("out", [T // 4, D], F32, kind="ExternalOutput")

    with tile.TileContext(nc) as tc:
        with (
            tc.tile_pool(name="singles", bufs=1) as singles,
            tc.tile_pool(name="persist", bufs=1) as persist,
            tc.tile_pool(name="dram", bufs=1, space="DRAM") as dram,
        ):
            psum = tc.alloc_tile_pool(name="psum1", bufs=2, space="PSUM")
            # ---------------- constants ----------------
            ident_f = singles.tile([P, P], F32)
            make_identity(nc, ident_f[:])
            # identity stacked twice vertically so slices at partition offsets
            # 0 and 64 both see a [64,64] identity (matmul requires matching
            # base partitions between lhsT and rhs)
            ident_m = singles.tile([P, HD], MM_DT)
            make_identity(nc, ident_m[0:HD, :])
            nc.sync.dma_start(out=ident_m[HD:P, :], in_=ident_m[0:HD, :])
            maskA = singles.tile([C, C], F32)  # upper-tri incl diag (keep i>=j in [j,i])
            make_upper_triangular(nc, maskA[:], val=1.0, diag=True)
            ones_f = singles.tile([P, C], F32)
            nc.vector.memset(ones_f[:], 1.0)
            ones_m = singles.tile([P, 1], MM_DT)
            nc.vector.memset(ones_m[:], 1.0)
            one1_f = singles.tile([1, 1], F32)
            nc.vector.memset(one1_f[:], 1.0)
            sel_sb = singles.tile([P, 2], F32)
            nc.gpsimd.dma_start(
                out=sel_sb[:],
                in_=bass.AP(tensor=sel.ap().tensor, offset=0,
                            ap=[[0, P], [1, 2]]),
            )

            # ---------------- weights ----------------
            wqf_sb = persist.tile([P, KT, 2 * HCOL], MM_DT)
            nc.gpsimd.dma_start(out=wqf_sb[:, :, 0:HCOL],
                                in_=wq.ap().rearrange("(ko p) m -> p ko m", p=P))
            nc.gpsimd.dma_start(out=wqf_sb[:, :, HCOL:2 * HCOL],
                                in_=wf.ap().rearrange("(ko p) m -> p ko m", p=P))
            wv_sb = persist.tile([P, KT, HCOL], MM_DT)
            nc.gpsimd.dma_start(out=wv_sb[:],
                                in_=wv.ap().rearrange("(ko p) m -> p ko m", p=P))
            wpo_sb = persist.tile([P, KT, D], MM_DT)
            nc.gpsimd.dma_start(out=wpo_sb[:],
                                in_=wpo.ap().rearrange("(ko p) c -> p ko c", p=P))
            csum_sb = singles.tile([1, D], MM_DT)
            nc.gpsimd.dma_start(out=csum_sb[:], in_=csum.ap())

            # colsum broadcast to all partitions: [P, D] f32 via K=1 matmul
            ones_1x = singles.tile([1, P], MM_DT)
            nc.vector.memset(ones_1x[:], 1.0)
            csum128 = singles.tile([P, D], F32)
            for n in range(2):
                ps_b = psum.tile([P, 512], F32, tag="bcast")
                nc.tensor.matmul(ps_b[:], ones_1x[:], csum_sb[0:1, ts(n, 512)],
                                 start=True, stop=True)
                nc.scalar.copy(csum128[:, ts(n, 512)], ps_b[:])

            # ---------------- persistent activations ----------------
            qd_T = persist.tile([P, 2, T], MM_DT)   # silu(q)*exp(bc), transposed
            ke_T = persist.tile([P, 2, T], MM_DT)   # (1-f)*exp(-bc), transposed
            v_nat = persist.tile([P, NCH, HCOL], MM_DT)  # v natural [tok, col]
            e_last = persist.tile([P, 2, NCH], F32)  # exp(bc_last) per chunk
            out_T = persist.tile([P, 2, T], F32)     # RNN output, transposed

            # ============ Stage 1: x^T, in_proj, decay ============
            xv = xl.ap()
            sb1 = tc.alloc_tile_pool(name="sb1", bufs=2)
            sb1s = tc.alloc_tile_pool(name="sb1s", bufs=2)
            for sb in range(NSB):
                x_nat = sb1.tile([P, 4, D], F32, tag="x_nat")
                nc.sync.dma_start(
                    out=x_nat[:],
                    in_=xv[ds(sb * SBT, SBT), :].rearrange("(a p) k -> p a k", p=P))
                xT = sb1.tile([P, KT, SBT], MM_DT, tag="xT")
                for a in range(4):
                    for k in range(KT):
                        ps_t = psum.tile([P, P], F32, tag="tr")
                        nc.tensor.transpose(ps_t[:], x_nat[:, a, ts(k, P)],
                                            ident_f[:])
                        nc.any.tensor_copy(out=xT[:, k, ts(a, P)], in_=ps_t[:])

                # (ii) q,f -> transposed layout [m, t]
                qf_sb = {}
                for m in range(4):
                    ps_qf = psum.tile([P, SBT], F32, tag="qf")
                    for k in range(KT):
                        nc.tensor.matmul(ps_qf[:], wqf_sb[:, k, ts(m, P)],
                                         xT[:, k, :], start=(k == 0),
                                         stop=(k == KT - 1))
                    t_sb = sb1s.tile([P, SBT], F32, tag=f"qf_{m}")
                    nc.scalar.activation(t_sb[:], ps_qf[:], AFT.Sigmoid)
                    if m < 2:  # silu(x) = x * sigmoid(x)
                        nc.vector.tensor_tensor(out=t_sb[:], in0=t_sb[:],
                                                in1=ps_qf[:], op=ALU.mult)
                    qf_sb[m] = t_sb

                for blk in range(2):
                    fs = qf_sb[2 + blk]
                    # lf = log_sigmoid(f) = ln(sigmoid(f))  (<0); reference
                    # applies logsigmoid to the gate value f = sigmoid(raw)
                    s2 = sb1s.tile([P, SBT], F32, tag="s2")
                    nc.scalar.activation(s2[:], fs[:], AFT.Sigmoid)
                    lf = sb1s.tile([P, SBT], F32, tag="lf")
                    nc.scalar.activation(lf[:], s2[:], AFT.Ln)
                    kk = sb1s.tile([P, SBT], F32, tag="kk")
                    nc.vector.tensor_scalar(out=kk[:], in0=fs[:], scalar1=-1.0,
                                            scalar2=1.0, op0=ALU.mult, op1=ALU.add)
                    # bc = within-chunk cumsum of lf (<= 0)
                    bc = sb1s.tile([P, SBT], F32, tag="bc")
                    for ch in range(SBT // C):
                        nc.vector.tensor_tensor_scan(
                            bc[:, ts(ch, C)], lf[:, ts(ch, C)], ones_f[:],
                            0.0, ALU.add, ALU.mult)
                    # e_last = exp(bc[last of chunk])
                    nc.scalar.activation(
                        e_last[:, blk, ds(sb * (SBT // C), SBT // C)],
                        bc.rearrange("p (t c) -> p t c", c=C)[:, :, C - 1],
                        AFT.Exp)
                    en = sb1s.tile([P, SBT], F32, tag="en")
                    nc.scalar.activation(en[:], bc[:], AFT.Exp, scale=-1.0)  # exp(-bc)
                    nc.vector.tensor_tensor(out=ke_T[:, blk, ds(sb * SBT, SBT)],
                                            in0=kk[:], in1=en[:], op=ALU.mult)
                    nc.scalar.activation(bc[:], bc[:], AFT.Exp)  # exp(bc) in place
                    nc.vector.tensor_tensor(out=qd_T[:, blk, ds(sb * SBT, SBT)],
                                            in0=qf_sb[blk][:], in1=bc[:],
                                            op=ALU.mult)

                # (i) v -> natural layout [tok, col]
                for a in range(4):
                    ps_v = psum.tile([P, HCOL], F32, tag="v")
                    for k in range(KT):
                        nc.tensor.matmul(ps_v[:], xT[:, k, ts(a, P)],
                                         wv_sb[:, k, :], start=(k == 0),
                                         stop=(k == KT - 1))
                    nc.any.tensor_copy(out=v_nat[:, sb * 4 + a, :], in_=ps_v[:])

            sb1s.release()
            sb1.release()
            psum.release()
            psum = tc.alloc_tile_pool(name="psum3", bufs=2, space="PSUM")

            # ============ Stage 3: chunked RNN ============
            rnn = tc.alloc_tile_pool(name="rnn", bufs=3)
            # per-head state; allocated full-height so the active slice sits at
            # the same base partition as qd/ke slices (matmul requirement)
            S_fa = [singles.tile([P, HD], F32, tag=f"S_f{h}", name=f"S_f{h}")
                    for h in range(NH)]
            S_ma = [singles.tile([P, HD], MM_DT, tag=f"S_m{h}", name=f"S_m{h}")
                    for h in range(NH)]
            S_f = [S_fa[h][(h % 2) * HD:(h % 2) * HD + HD, :] for h in range(NH)]
            S_m = [S_ma[h][(h % 2) * HD:(h % 2) * HD + HD, :] for h in range(NH)]
            for h in range(NH):
                nc.vector.memset(S_f[h], 0.0)
                nc.vector.memset(S_m[h], 0.0)

            for t in range(NCH):
                for h in range(NH):
                    blk, row = h // 2, (h % 2) * HD
                    qd = qd_T[row:row + HD, blk, ts(t, C)]
                    ke = ke_T[row:row + HD, blk, ts(t, C)]
                    vv = v_nat[:, t, ts(h, HD)]

                    # A^T[j,i] = sum_d ke[j,d] qd[i,d]  (then causal mask i>=j)
                    ps_A = psum.tile([C, C], F32, tag="A")
                    nc.tensor.matmul(ps_A[:], ke, qd, start=True, stop=True)
                    A_sb = rnn.tile([C, C], MM_DT, tag="A_sb")
                    nc.vector.tensor_tensor(out=A_sb[:], in0=ps_A[:],
                                            in1=maskA[:], op=ALU.mult)

                    # o^T[d,i] = S^T@qd^T + v^T@A^T  -> [HD, C]
                    ps_o = psum.tile([HD, C], F32, tag="o")
                    if t > 0:
                        nc.tensor.matmul(ps_o[:], S_m[h], qd, start=True,
                                         stop=False)
                        nc.tensor.matmul(ps_o[:], vv, A_sb[:], start=False,
                                         stop=True)
                    else:
                        nc.tensor.matmul(ps_o[:], vv, A_sb[:], start=True,
                                         stop=True)
                    nc.scalar.copy(out_T[row:row + HD, blk, ts(t, C)], ps_o[:])

                    if t < NCH - 1:
                        # ke natural via PE transpose, then G = ke^T_nat @ v
                        ps_kt = psum.tile([C, HD], MM_DT, tag="kt")
                        nc.tensor.transpose(ps_kt[:], ke,
                                            ident_m[row:row + HD, :])
                        ke_n = rnn.tile([C, HD], MM_DT, tag="ke_n")
                        nc.any.tensor_copy(out=ke_n[:], in_=ps_kt[:])
                        ps_G = psum.tile([HD, HD], F32, tag="G")
                        nc.tensor.matmul(ps_G[:], ke_n[:], vv, start=True,
                                         stop=True)
                        # S = e_last * (S + G)
                        nc.vector.tensor_tensor(out=S_f[h], in0=S_f[h],
                                                in1=ps_G[:], op=ALU.add)
                        nc.vector.tensor_scalar_mul(
                            S_f[h], S_f[h],
                            e_last[row:row + HD, blk, t:t + 1])
                        nc.vector.tensor_copy(S_m[h], S_f[h])

            rnn.release()
            psum.release()

            # ============ Stage 4: AllToAll ============
            a2a_in = dram.tile([N_CORES, HCOL, T // 4], F32)
            a2a_out = dram.tile([N_CORES, HCOL, T // 4], F32)
            for p in range(N_CORES):
                for blk in range(2):
                    nc.sync.dma_start(
                        out=a2a_in[p, ds(blk * P, P), :],
                        in_=out_T[:, blk, ds((p % 4) * (T // 4), T // 4)])
            nc.gpsimd.collective_compute(
                "AllToAll", ALU.bypass, replica_groups=GROUPS8,
                ins=[a2a_in[:].opt()], outs=[a2a_out[:].opt()])

            # ============ Stage 5: layernorm + out_proj ============
            fin = tc.alloc_tile_pool(name="fin", bufs=1)
            fin2 = tc.alloc_tile_pool(name="fin2", bufs=3)
            psum = tc.alloc_tile_pool(name="psum5", bufs=2, space="PSUM")
            TT = T // 4  # 1024 tokens
            ylo = fin.tile([P, KT, TT], MM_DT, tag="ylo")
            yhi = fin.tile([P, KT, TT], MM_DT, tag="yhi")
            nc.gpsimd.dma_start(
                out=ylo[:],
                in_=a2a_out[0:4].rearrange("g (q p) t -> p (g q) t", p=P))
            nc.gpsimd.dma_start(
                out=yhi[:],
                in_=a2a_out[4:8].rearrange("g (q p) t -> p (g q) t", p=P))
            yT = fin.tile([P, KT, TT], MM_DT, tag="yT")
            nc.vector.tensor_scalar_mul(yT[:], ylo[:], sel_sb[:, 0:1])
            nc.vector.tensor_scalar_mul(ylo[:], yhi[:], sel_sb[:, 1:2])
            nc.vector.tensor_tensor(out=yT[:], in0=yT[:], in1=ylo[:], op=ALU.add)
            ysq = fin.tile([P, KT, TT], MM_DT, tag="ysq")
            nc.vector.tensor_tensor(out=ysq[:], in0=yT[:], in1=yT[:], op=ALU.mult)

            # stats: column sums over D via ones-matmuls -> [1, TT] f32
            ssum = fin.tile([1, TT], F32, tag="ssum")
            ssq = fin.tile([1, TT], F32, tag="ssq")
            for n in range(TT // 512):
                ps_s = psum.tile([1, 512], F32, tag="stat")
                for k in range(KT):
                    nc.tensor.matmul(ps_s[:], ones_m[:], yT[:, k, ts(n, 512)],
                                     start=(k == 0), stop=(k == KT - 1))
                nc.vector.tensor_copy(ssum[:, ts(n, 512)], ps_s[:])
                ps_q = psum.tile([1, 512], F32, tag="stat")
                for k in range(KT):
                    nc.tensor.matmul(ps_q[:], ones_m[:], ysq[:, k, ts(n, 512)],
                                     start=(k == 0), stop=(k == KT - 1))
                nc.vector.tensor_copy(ssq[:, ts(n, 512)], ps_q[:])

            # transpose stats to token-partition layout [P, TT//P] via K=1 f32 mms
            muT = fin.tile([P, TT // P], F32, tag="muT")
            sqT = fin.tile([P, TT // P], F32, tag="sqT")
            for a in range(TT // P):
                ps_t1 = psum.tile([P, 1], F32, tag="stt")
                nc.tensor.matmul(ps_t1[:], ssum[0:1, ts(a, P)], one1_f[:],
                                 start=True, stop=True)
                nc.vector.tensor_copy(muT[:, a:a + 1], ps_t1[:])
                ps_t2 = psum.tile([P, 1], F32, tag="stt")
                nc.tensor.matmul(ps_t2[:], ssq[0:1, ts(a, P)], one1_f[:],
                                 start=True, stop=True)
                nc.vector.tensor_copy(sqT[:, a:a + 1], ps_t2[:])

            # mu = sum/D ; var = sq/D - mu^2 ; rstd = 1/sqrt(var+eps); aT = mu*rstd
            nc.vector.tensor_scalar_mul(muT[:], muT[:], 1.0 / D)
            nc.vector.tensor_scalar_mul(sqT[:], sqT[:], 1.0 / D)
            msq = fin.tile([P, TT // P], F32, tag="msq")
            nc.vector.tensor_tensor(out=msq[:], in0=muT[:], in1=muT[:], op=ALU.mult)
            nc.vector.tensor_tensor(out=sqT[:], in0=sqT[:], in1=msq[:],
                                    op=ALU.subtract)
            eps_sb = singles.tile([P, 1], F32)
            nc.vector.memset(eps_sb[:], EPS)
            nc.scalar.activation(sqT[:], sqT[:], AFT.Sqrt, bias=eps_sb[:])
            rstd = fin.tile([P, TT // P], F32, tag="rstd")
            nc.vector.reciprocal(rstd[:], sqT[:])
            aT = fin.tile([P, TT // P], F32, tag="aT")
            nc.vector.tensor_tensor(out=aT[:], in0=muT[:], in1=rstd[:], op=ALU.mult)

            # final: out[t,c] = rstd[t]*(y^T W')[t,c] - (rstd[t]*mu[t])*colsum[c]
            for a in range(TT // P):
                for n in range(2):
                    ps_f = psum.tile([P, 512], F32, tag="fin")
                    for k in range(KT):
                        nc.tensor.matmul(ps_f[:], yT[:, k, ts(a, P)],
                                         wpo_sb[:, k, ts(n, 512)],
                                         start=(k == 0), stop=(k == KT - 1))
                    f_sb = fin2.tile([P, 512], F32, tag="f_sb")
                    nc.vector.tensor_scalar_mul(f_sb[:], ps_f[:], rstd[:, a:a + 1])
                    c_sb = fin2.tile([P, 512], F32, tag="c_sb")
                    nc.vector.tensor_scalar_mul(c_sb[:], csum128[:, ts(n, 512)],
                                                aT[:, a:a + 1])
                    nc.vector.tensor_tensor(out=f_sb[:], in0=f_sb[:], in1=c_sb[:],
                                            op=ALU.subtract)
                    nc.sync.dma_start(out=out.ap()[ds(a * P, P), ts(n, 512)],
                                      in_=f_sb[:])

            fin2.release()
            fin.release()
            psum.release()

    nc.compile()
    return nc


_NC_CACHE = None


def _get_program():
    global _NC_CACHE
    if _NC_CACHE is None:
        _NC_CACHE = build_program()
    return _NC_CACHE


def kernel(x, W_in, W_out, norm_w, _want_results=False, _trace=False):
    x = np.asarray(x, dtype=np.float32)
    W_in = np.asarray(W_in, dtype=np.float32)
    W_out = np.asarray(W_out, dtype=np.float32)
    norm_w = np.asarray(norm_w, dtype=np.float32)

    wpo = (norm_w[:, None] * W_out).astype(np.float32)
    csum = wpo.sum(axis=0, keepdims=True).astype(np.float32)

    in_maps = []
    for c in range(N_CORES):
        bi, hg = c // 4, c % 4
        m = {
            "xl": np.ascontiguousarray(x[bi]),
            "wq": np.ascontiguousarray(W_in[:, hg * HCOL:(hg + 1) * HCOL]),
            "wf": np.ascontiguousarray(W_in[:, D + hg * HCOL:D + (hg + 1) * HCOL]),
            "wv": np.ascontiguousarray(W_in[:, 2 * D + hg * HCOL:2 * D + (hg + 1) * HCOL]),
            "wpo": wpo,
            "csum": csum,
            "sel": np.array([[1.0, 0.0]] if bi == 0 else [[0.0, 1.0]],
                            dtype=np.float32),
        }
        in_maps.append(m)

    nc = _get_program()
    res = run_bass_kernel_spmd(nc, in_maps, core_ids=list(range(N_CORES)),
                               trace=_trace)
    outf = np.empty((2, T, D), dtype=np.float32)
    for c in range(N_CORES):
        bi, rank = c // 4, c % 4
        outf[bi, rank * (T // 4):(rank + 1) * (T // 4), :] = res.results[c]["out"]
    if _want_results:
        return outf, res
    return outf
